# revision 11
# baseline (speedup 1.0000x reference)
"""Causal GQA self-attention with RoPE for TRN2, 8 NeuronCores.

Problem: B=2, S=2048, D=2048, H=16 q-heads, KV=4 kv-heads, HD=128.

Sharding: core c = (batch b = c//4, kv-group g = c%4). Each core computes
q-heads 4g..4g+3 and kv-head g for batch b:
  qT = (x[b] @ wq[:, 512g:512g+512]).T      via PE, transposed layout
  kT/vT likewise (128-wide slices), RoPE on qT/kT,
  flash-style causal attention in the S^T (keys-on-partitions) layout,
  partial^T = (attn @ wo[512g:512g+512, :]).T
Host sums the 4 partials per batch and transposes back.

Matmuls in fp16 (scores/output accumulate fp32 in PSUM, softmax
denominator accumulated fp32); end-to-end rel err ~4e-4 vs fp32.
"""
import sys

sys.path.insert(0, "/opt/trn_rl_repo")

import numpy as np

import concourse.bass as bass
import concourse.tile as tile
from concourse import bacc, mybir
from concourse.bass_utils import run_bass_kernel_spmd

F32 = mybir.dt.float32
F16 = mybir.dt.float16
AF = mybir.ActivationFunctionType
OP = mybir.AluOpType

P = 128          # partitions / head dim
S = 2048         # sequence length
D = 2048         # model dim
NH = 4           # q heads per core
QW = NH * P      # q projection width per core (512)
NKD = D // P     # contraction chunks (16)
QCH = 512        # query chunk (free dim of attention matmuls)
NQC = S // QCH   # 4
KCH = P          # key chunk (128, on partitions)
NKC = S // KCH   # 16
SCALE = float(P) ** -0.5


def _host_constants():
    inv = 1.0 / (10000.0 ** (np.arange(0, P, 2, dtype=np.float64) / P))  # [64]
    pos = np.arange(S, dtype=np.float64)
    freqs = pos[:, None] * inv[None, :]                  # [S, 64]
    emb = np.concatenate([freqs, freqs], axis=-1)        # [S, 128]
    cosT = np.cos(emb).T.astype(np.float16).copy()       # [128, S]
    sinT = np.sin(emb).T.astype(np.float16)
    sinT[: P // 2] *= np.float16(-1.0)                   # fold rotate_half sign
    sinT = sinT.copy()
    # causal step masks: mask[p, j, q] = 1 if q >= p + 128*j
    q = np.arange(QCH)[None, None, :]
    p = np.arange(P)[:, None, None]
    j = np.arange(4)[None, :, None]
    masks = (q >= p + KCH * j).astype(np.float16)        # [128, 4, 512]
    ones = np.ones((P, 1), dtype=np.float32)
    return cosT, sinT, masks, ones


def build_nc():
    cosT_np, sinT_np, masks_np, ones_np = _host_constants()

    nc = bacc.Bacc(None)
    xT_d = nc.dram_tensor("xT", [D, S], F16, kind="ExternalInput")
    wq_d = nc.dram_tensor("wq", [D, QW], F16, kind="ExternalInput")
    wk_d = nc.dram_tensor("wk", [D, P], F16, kind="ExternalInput")
    wv_d = nc.dram_tensor("wv", [D, P], F16, kind="ExternalInput")
    wo_d = nc.dram_tensor("wo", [QW, D], F16, kind="ExternalInput")
    out_d = nc.dram_tensor("outT", [D, S], F32, kind="ExternalOutput")

    cos_d = nc.inline_tensor(cosT_np, name="cosT")
    sin_d = nc.inline_tensor(sinT_np, name="sinT")
    mask_d = nc.inline_tensor(masks_np, name="masks")
    ones_d = nc.inline_tensor(ones_np, name="onesv")

    # DRAM views with the contraction dim split for SBUF partitions.
    xT_v = xT_d[:].rearrange("(kd p) s -> p kd s", p=P)
    wq_v = wq_d[:].rearrange("(kd p) c -> p kd c", p=P)
    wk_v = wk_d[:].rearrange("(kd p) c -> p kd c", p=P)
    wv_v = wv_d[:].rearrange("(kd p) c -> p kd c", p=P)
    wo_v = wo_d[:].rearrange("(a p) o -> p a o", p=P)

    with tile.TileContext(nc) as tc:
        with tc.tile_pool(name="persist", bufs=1) as pp:
            qT = pp.tile([P, NH, S], F16)        # q^T; attention overwrites
            kT = pp.tile([P, S], F16)
            vT = pp.tile([P, S], F16)
            vK = pp.tile([P, NKC, P], F16)       # V as (kpos, kchunk, hd)
            mask_t = pp.tile([P, 4, QCH], F16)
            ones_t = pp.tile([P, 1], F32)
            nc.sync.dma_start(mask_t[:], mask_d[:])
            nc.sync.dma_start(ones_t[:], ones_d[:])

            # attention output overwrites qT in place: slice (h, jq-chunk) is
            # written only after every read of that same slice is done.
            attnT = qT

            # ---------------- Phase 1: QKV projections -----------------
            with tc.tile_pool(name="xqp", bufs=2) as xqp, \
                 tc.tile_pool(name="p1", bufs=2) as p1, \
                 tc.tile_pool(name="p1c", bufs=1) as p1c, \
                 tc.tile_pool(name="psA", bufs=1, space="PSUM") as psA:
                cos_t = p1c.tile([P, S], F16)
                sin_t = p1c.tile([P, S], F16)
                nc.sync.dma_start(cos_t[:], cos_d[:])
                nc.sync.dma_start(sin_t[:], sin_d[:])

                # cc order: k, v, then q heads — k/v ready earliest.
                CC = [("k", 0), ("v", 0), ("q", 0), ("q", 1), ("q", 2), ("q", 3)]
                for iq in range(NQC):            # 512-row quarters
                    xq = xqp.tile([P, NKD, QCH], F16, tag="xq")
                    for kd in range(NKD):
                        nc.sync.dma_start(
                            xq[:, kd, :], xT_v[:, kd, iq * QCH : (iq + 1) * QCH]
                        )
                    for icc, (kind, hh) in enumerate(CC):
                        if kind == "q":
                            w_v = wq_v[:, :, hh * P : (hh + 1) * P]
                        elif kind == "k":
                            w_v = wk_v
                        else:
                            w_v = wv_v
                        wt = p1.tile([P, NKD, P], F16, tag="wt")
                        nc.sync.dma_start(wt[:], w_v)
                        ps = psA.tile([P, QCH], F32, tag=f"proj{icc % 2}")
                        for kd in range(NKD):
                            nc.tensor.matmul(
                                ps[:],
                                wt[:, kd, :],
                                xq[:, kd, :],
                                start=(kd == 0),
                                stop=(kd == NKD - 1),
                            )
                        if kind == "q":
                            dst = qT[:, hh, iq * QCH : (iq + 1) * QCH]
                        elif kind == "k":
                            dst = kT[:, iq * QCH : (iq + 1) * QCH]
                        else:
                            dst = vT[:, iq * QCH : (iq + 1) * QCH]
                        nc.scalar.copy(out=dst, in_=ps[:])

                # ---------------- RoPE (k then per-head q) --------------
                def rope(dst_ap):
                    sw = p1.tile([P, S], F16, tag="swap")
                    half = P // 2
                    nc.sync.dma_start(sw[:half, :], dst_ap[half:, :])
                    nc.sync.dma_start(sw[half:, :], dst_ap[:half, :])
                    nc.vector.tensor_tensor(sw[:], sw[:], sin_t[:], OP.mult)
                    nc.vector.tensor_tensor(dst_ap, dst_ap, cos_t[:], OP.mult)
                    nc.vector.tensor_tensor(dst_ap, dst_ap, sw[:], OP.add)

                rope(kT[:])
                for hh in range(NH):
                    rope(qT[:, hh, :])

                # ---------------- V transpose (DMA, fp16) ---------------
                nc.sync.dma_start_transpose(vK[:], vT[:])

                # ---------------- Phase 2: attention --------------------
                with tc.tile_pool(name="p2", bufs=4) as p2, \
                     tc.tile_pool(name="p2b", bufs=1) as p2b:
                    dall = p2b.tile([1, NH * S], F32)   # all 16 denominators
                    for h in range(NH):
                        for jq in range(NQC):
                            nkc = 4 * (jq + 1)
                            qs = qT[:, h, jq * QCH : (jq + 1) * QCH]
                            ops = psA.tile([P, QCH], F32, tag="o")
                            acc = p2.tile([P, QCH], F32, tag="acc")
                            npair = nkc // 2
                            for ip in range(npair):
                                kc0 = 2 * ip
                                sps = psA.tile([P, 2 * QCH], F32, tag=f"s{ip % 2}")
                                for k2 in range(2):
                                    nc.tensor.matmul(
                                        sps[:, k2 * QCH : (k2 + 1) * QCH],
                                        kT[:, (kc0 + k2) * P : (kc0 + k2 + 1) * P],
                                        qs,
                                        start=True,
                                        stop=True,
                                    )
                                pT = p2.tile([P, 2 * QCH], F16, tag="pT")
                                nc.scalar.activation(
                                    pT[:], sps[:], AF.Exp, scale=SCALE
                                )
                                jd0 = kc0 - 4 * jq
                                if jd0 >= 0:
                                    # both halves on the diagonal band
                                    nc.vector.tensor_tensor(
                                        pT[:],
                                        pT[:],
                                        mask_t[:, jd0 : jd0 + 2, :],
                                        OP.mult,
                                    )
                                elif jd0 == -1:
                                    nc.vector.tensor_tensor(
                                        pT[:, QCH:],
                                        pT[:, QCH:],
                                        mask_t[:, 0, :],
                                        OP.mult,
                                    )
                                for k2 in range(2):
                                    kc = kc0 + k2
                                    pslice = pT[:, k2 * QCH : (k2 + 1) * QCH]
                                    if kc == 0:
                                        nc.gpsimd.tensor_copy(
                                            out=acc[:], in_=pslice
                                        )
                                    else:
                                        nc.gpsimd.tensor_tensor(
                                            acc[:], acc[:], pslice, OP.add
                                        )
                                    nc.tensor.matmul(
                                        ops[:],
                                        vK[:, kc, :],
                                        pslice,
                                        start=(kc == 0),
                                        stop=(kc == nkc - 1),
                                    )
                            dps = psA.tile([1, QCH], F32, tag="d")
                            nc.tensor.matmul(
                                dps[:], ones_t[:], acc[:], start=True, stop=True
                            )
                            idx = h * NQC + jq
                            nc.vector.tensor_copy(
                                out=dall[:, idx * QCH : (idx + 1) * QCH],
                                in_=dps[:],
                            )
                            # park unnormalized O^T; normalized later in bulk
                            nc.scalar.copy(
                                out=attnT[:, h, jq * QCH : (jq + 1) * QCH],
                                in_=ops[:],
                            )

                    # batched normalization: one reciprocal + one broadcast
                    dinv = p2b.tile([1, NH * S], F16, tag="dinv", name="dinv")
                    with nc.allow_low_precision(
                        reason="1/D rounded to fp16 once; ~2e-4 on output"
                    ):
                        nc.vector.reciprocal(dinv[:], dall[:])
                    dib = p2b.tile([P, NH * S], F16, tag="dib")
                    nc.gpsimd.partition_broadcast(dib[:], dinv[:])
                    for h in range(NH):
                        for jq in range(NQC):
                            idx = h * NQC + jq
                            nc.vector.tensor_tensor(
                                attnT[:, h, jq * QCH : (jq + 1) * QCH],
                                attnT[:, h, jq * QCH : (jq + 1) * QCH],
                                dib[:, idx * QCH : (idx + 1) * QCH],
                                OP.mult,
                            )

            # ---------------- Phase 3: output projection ----------------
            with tc.tile_pool(name="p3", bufs=4) as p3, \
                 tc.tile_pool(name="p3w", bufs=1) as p3w, \
                 tc.tile_pool(name="psB", bufs=1, space="PSUM") as psB:
                wo_t = p3w.tile([P, NH, D], F16)
                nc.sync.dma_start(wo_t[:], wo_v)
                for oc in range(D // P):
                    pos = [
                        psB.tile([P, QCH], F32, tag=f"po{jq}", name=f"po{jq}")
                        for jq in range(NQC)
                    ]
                    for a in range(NH):
                        for jq in range(NQC):
                            nc.tensor.matmul(
                                pos[jq][:],
                                wo_t[:, a, oc * P : (oc + 1) * P],
                                attnT[:, a, jq * QCH : (jq + 1) * QCH],
                                start=(a == 0),
                                stop=(a == NH - 1),
                            )
                    for jq in range(NQC):
                        ot = p3.tile([P, QCH], F32, tag="ot")
                        if jq % 2 == 0:
                            nc.scalar.copy(out=ot[:], in_=pos[jq][:])
                        else:
                            nc.vector.tensor_copy(out=ot[:], in_=pos[jq][:])
                        nc.sync.dma_start(
                            out_d[oc * P : (oc + 1) * P, jq * QCH : (jq + 1) * QCH],
                            ot[:],
                        )

    nc.finalize()
    return nc


_NC = None


def _get_nc():
    global _NC
    if _NC is None:
        _NC = build_nc()
    return _NC


def make_in_maps(x, wq, wk, wv, wo):
    x = np.asarray(x, dtype=np.float32)
    f16 = np.float16
    in_maps = []
    for c in range(8):
        b, g = c // 4, c % 4
        in_maps.append(
            {
                "xT": np.ascontiguousarray(x[b].T).astype(f16),
                "wq": np.asarray(wq[:, QW * g : QW * (g + 1)], dtype=f16),
                "wk": np.asarray(wk[:, P * g : P * (g + 1)], dtype=f16),
                "wv": np.asarray(wv[:, P * g : P * (g + 1)], dtype=f16),
                "wo": np.asarray(wo[QW * g : QW * (g + 1), :], dtype=f16),
            }
        )
    return in_maps


def kernel(x, wq, wk, wv, wo):
    nc = _get_nc()
    in_maps = make_in_maps(x, wq, wk, wv, wo)
    res = run_bass_kernel_spmd(nc, in_maps, list(range(8)))
    parts = [res.results[c]["outT"] for c in range(8)]
    out = np.stack(
        [
            (parts[0] + parts[1] + parts[2] + parts[3]).T,
            (parts[4] + parts[5] + parts[6] + parts[7]).T,
        ]
    ).astype(np.float32)
    return out


# revision 13
# speedup vs baseline: 1.1914x; 1.1914x over previous
"""Causal GQA self-attention with RoPE for TRN2, 8 NeuronCores.

Problem: B=2, S=2048, D=2048, H=16 q-heads, KV=4 kv-heads, HD=128.

Sharding: core c = (batch b = c//4, kv-group g = c%4). Each core computes
q-heads 4g..4g+3 and kv-head g for batch b:
  qT = (x[b] @ wq[:, 512g:512g+512]).T      via PE, transposed layout
  kT/vT likewise (128-wide slices), RoPE on qT/kT,
  flash-style causal attention in the S^T (keys-on-partitions) layout,
  partial^T = (attn @ wo[512g:512g+512, :]).T
Host sums the 4 partials per batch and transposes back.

Matmuls in fp16 (scores/output accumulate fp32 in PSUM, softmax
denominator accumulated fp32); end-to-end rel err ~5e-4 vs fp32.
"""
import sys

sys.path.insert(0, "/opt/trn_rl_repo")

import numpy as np

import concourse.bass as bass
import concourse.tile as tile
from concourse import bacc, mybir
from concourse.bass_utils import run_bass_kernel_spmd

F32 = mybir.dt.float32
F16 = mybir.dt.float16
AF = mybir.ActivationFunctionType
OP = mybir.AluOpType

P = 128          # partitions / head dim
S = 2048         # sequence length
D = 2048         # model dim
NH = 4           # q heads per core
QW = NH * P      # q projection width per core (512)
NKD = D // P     # contraction chunks (16)
QCH = 512        # query chunk (free dim of attention matmuls)
NQC = S // QCH   # 4
KCH = P          # key chunk (128, on partitions)
NKC = S // KCH   # 16
SCALE = float(P) ** -0.5


def _host_constants():
    inv = 1.0 / (10000.0 ** (np.arange(0, P, 2, dtype=np.float64) / P))  # [64]
    pos = np.arange(S, dtype=np.float64)
    freqs = pos[:, None] * inv[None, :]                  # [S, 64]
    emb = np.concatenate([freqs, freqs], axis=-1)        # [S, 128]
    cosT = np.cos(emb).T.astype(np.float16).copy()       # [128, S]
    sinT = np.sin(emb).T.astype(np.float16)
    sinT[: P // 2] *= np.float16(-1.0)                   # fold rotate_half sign
    sinT = sinT.copy()
    # causal step masks: mask[p, j, q] = 1 if q >= p + 128*j
    q = np.arange(QCH)[None, None, :]
    p = np.arange(P)[:, None, None]
    j = np.arange(4)[None, :, None]
    masks = (q >= p + KCH * j).astype(np.float16)        # [128, 4, 512]
    ones = np.ones((P, P), dtype=np.float32)
    return cosT, sinT, masks, ones


def build_nc():
    cosT_np, sinT_np, masks_np, ones_np = _host_constants()

    nc = bacc.Bacc(None)
    xT_d = nc.dram_tensor("xT", [D, S], F16, kind="ExternalInput")
    wq_d = nc.dram_tensor("wq", [D, QW], F16, kind="ExternalInput")
    wk_d = nc.dram_tensor("wk", [D, P], F16, kind="ExternalInput")
    wv_d = nc.dram_tensor("wv", [D, P], F16, kind="ExternalInput")
    wo_d = nc.dram_tensor("wo", [QW, D], F16, kind="ExternalInput")
    out_d = nc.dram_tensor("outT", [D, S], F32, kind="ExternalOutput")

    cos_d = nc.inline_tensor(cosT_np, name="cosT")
    sin_d = nc.inline_tensor(sinT_np, name="sinT")
    mask_d = nc.inline_tensor(masks_np, name="masks")
    ones_d = nc.inline_tensor(ones_np, name="onesm")

    # DRAM views with the contraction dim split for SBUF partitions.
    xT_v = xT_d[:].rearrange("(kd p) s -> p kd s", p=P)
    wq_v = wq_d[:].rearrange("(kd p) c -> p kd c", p=P)
    wk_v = wk_d[:].rearrange("(kd p) c -> p kd c", p=P)
    wv_v = wv_d[:].rearrange("(kd p) c -> p kd c", p=P)
    wo_v = wo_d[:].rearrange("(a p) o -> p a o", p=P)

    with tile.TileContext(nc) as tc:
        with tc.tile_pool(name="persist", bufs=1) as pp:
            qT = pp.tile([P, NH, S], F16)        # q^T; attention overwrites
            kT = pp.tile([P, S], F16)
            vT = pp.tile([P, S], F16)
            vK = pp.tile([P, NKC, P], F16)       # V as (kpos, kchunk, hd)
            mask_t = pp.tile([P, 4, QCH], F16)
            ones_t = pp.tile([P, P], F32)        # all-ones: colsum + broadcast
            nc.sync.dma_start(mask_t[:], mask_d[:])
            nc.sync.dma_start(ones_t[:], ones_d[:])

            # attention output overwrites qT in place: slice (h, jq-chunk) is
            # written only after every read of that same slice is done.
            attnT = qT

            # ======== Phase 1: QKV projections (full xT resident) ========
            with tc.tile_pool(name="xp", bufs=1) as xp, \
                 tc.tile_pool(name="p1", bufs=2) as p1, \
                 tc.tile_pool(name="p1c", bufs=1) as p1c, \
                 tc.tile_pool(name="psP", bufs=1, space="PSUM") as psP:
                cos_t = p1c.tile([P, S], F16)
                sin_t = p1c.tile([P, S], F16)
                nc.sync.dma_start(cos_t[:], cos_d[:])
                nc.sync.dma_start(sin_t[:], sin_d[:])

                def rope(dst_ap):
                    sw = p1.tile([P, S], F16, tag="swap")
                    half = P // 2
                    nc.sync.dma_start(sw[:half, :], dst_ap[half:, :])
                    nc.sync.dma_start(sw[half:, :], dst_ap[:half, :])
                    nc.vector.tensor_tensor(sw[:], sw[:], sin_t[:], OP.mult)
                    nc.vector.tensor_tensor(dst_ap, dst_ap, cos_t[:], OP.mult)
                    nc.vector.tensor_tensor(dst_ap, dst_ap, sw[:], OP.add)

                xf = xp.tile([P, NKD, S], F16)
                for kd in range(NKD):
                    nc.sync.dma_start(xf[:, kd, :], xT_v[:, kd, :])

                # cc order: k, v, then q heads — k/v ready earliest.
                CC = [("k", 0), ("v", 0), ("q", 0), ("q", 1), ("q", 2), ("q", 3)]
                for icc, (kind, hh) in enumerate(CC):
                    if kind == "q":
                        w_v = wq_v[:, :, hh * P : (hh + 1) * P]
                    elif kind == "k":
                        w_v = wk_v
                    else:
                        w_v = wv_v
                    wt = p1.tile([P, NKD, P], F16, tag="wt")
                    nc.sync.dma_start(wt[:], w_v)
                    for jr in range(NQC):
                        ps = psP.tile([P, QCH], F32, tag=f"pj{jr % 2}")
                        for kd in range(NKD):
                            nc.tensor.matmul(
                                ps[:],
                                wt[:, kd, :],
                                xf[:, kd, jr * QCH : (jr + 1) * QCH],
                                start=(kd == 0),
                                stop=(kd == NKD - 1),
                            )
                        if kind == "q":
                            dst = qT[:, hh, jr * QCH : (jr + 1) * QCH]
                        elif kind == "k":
                            dst = kT[:, jr * QCH : (jr + 1) * QCH]
                        else:
                            dst = vT[:, jr * QCH : (jr + 1) * QCH]
                        nc.scalar.copy(out=dst, in_=ps[:])
                    if kind == "k":
                        rope(kT[:])
                    elif kind == "v":
                        nc.sync.dma_start_transpose(vK[:], vT[:])
                    else:
                        rope(qT[:, hh, :])

                # ======== Phase 2: attention (overlaps phase-1 tail) ======
                with tc.tile_pool(name="p2", bufs=4) as p2, \
                     tc.tile_pool(name="psAtt", bufs=1, space="PSUM") as psAtt:
                    for h in range(NH):
                        for jq in range(NQC - 1, -1, -1):
                            nkc = 4 * (jq + 1)
                            qs = qT[:, h, jq * QCH : (jq + 1) * QCH]
                            ops = psAtt.tile([P, QCH], F32, tag="o")
                            acc = p2.tile([P, QCH], F32, tag="acc")
                            npair = nkc // 2
                            for ip in range(npair):
                                kc0 = 2 * ip
                                sps = psAtt.tile(
                                    [P, 2 * QCH], F32, tag=f"s{ip % 2}"
                                )
                                for k2 in range(2):
                                    nc.tensor.matmul(
                                        sps[:, k2 * QCH : (k2 + 1) * QCH],
                                        kT[:, (kc0 + k2) * P : (kc0 + k2 + 1) * P],
                                        qs,
                                        start=True,
                                        stop=True,
                                    )
                                pT = p2.tile([P, 2 * QCH], F16, tag="pT")
                                nc.scalar.activation(
                                    pT[:], sps[:], AF.Exp, scale=SCALE
                                )
                                jd0 = kc0 - 4 * jq
                                if jd0 >= 0:
                                    # both halves on the diagonal band
                                    nc.vector.tensor_tensor(
                                        pT[:],
                                        pT[:],
                                        mask_t[:, jd0 : jd0 + 2, :],
                                        OP.mult,
                                    )
                                for k2 in range(2):
                                    kc = kc0 + k2
                                    pslice = pT[:, k2 * QCH : (k2 + 1) * QCH]
                                    if kc == 0:
                                        nc.gpsimd.tensor_copy(
                                            out=acc[:], in_=pslice
                                        )
                                    else:
                                        nc.gpsimd.tensor_tensor(
                                            acc[:], acc[:], pslice, OP.add
                                        )
                                    nc.tensor.matmul(
                                        ops[:],
                                        vK[:, kc, :],
                                        pslice,
                                        start=(kc == 0),
                                        stop=(kc == nkc - 1),
                                    )
                            # denominator, broadcast to all partitions by the
                            # all-ones stationary matmul; then 1/x on DVE.
                            dps = psAtt.tile([P, QCH], F32, tag="d")
                            nc.tensor.matmul(
                                dps[:], ones_t[:], acc[:], start=True, stop=True
                            )
                            dib = p2.tile([P, QCH], F32, tag="dib")
                            nc.vector.reciprocal_approx_fast(dib[:], dps[:])
                            nc.vector.tensor_tensor(
                                attnT[:, h, jq * QCH : (jq + 1) * QCH],
                                ops[:],
                                dib[:],
                                OP.mult,
                            )

            # ======== Phase 3: output projection =========================
            with tc.tile_pool(name="p3", bufs=4) as p3, \
                 tc.tile_pool(name="p3w", bufs=1) as p3w, \
                 tc.tile_pool(name="psB", bufs=1, space="PSUM") as psB:
                wo_t = p3w.tile([P, NH, D], F16)
                nc.sync.dma_start(wo_t[:], wo_v)
                for oc in range(D // P):
                    pos = [
                        psB.tile([P, QCH], F32, tag=f"po{jq}", name=f"po{jq}")
                        for jq in range(NQC)
                    ]
                    for a in range(NH):
                        for jq in range(NQC):
                            nc.tensor.matmul(
                                pos[jq][:],
                                wo_t[:, a, oc * P : (oc + 1) * P],
                                attnT[:, a, jq * QCH : (jq + 1) * QCH],
                                start=(a == 0),
                                stop=(a == NH - 1),
                            )
                    for jq in range(NQC):
                        ot = p3.tile([P, QCH], F32, tag="ot")
                        if jq % 2 == 0:
                            nc.scalar.copy(out=ot[:], in_=pos[jq][:])
                        else:
                            nc.vector.tensor_copy(out=ot[:], in_=pos[jq][:])
                        nc.sync.dma_start(
                            out_d[oc * P : (oc + 1) * P, jq * QCH : (jq + 1) * QCH],
                            ot[:],
                        )

    nc.finalize()
    return nc


_NC = None


def _get_nc():
    global _NC
    if _NC is None:
        _NC = build_nc()
    return _NC


def make_in_maps(x, wq, wk, wv, wo):
    x = np.asarray(x, dtype=np.float32)
    f16 = np.float16
    in_maps = []
    for c in range(8):
        b, g = c // 4, c % 4
        in_maps.append(
            {
                "xT": np.ascontiguousarray(x[b].T).astype(f16),
                "wq": np.asarray(wq[:, QW * g : QW * (g + 1)], dtype=f16),
                "wk": np.asarray(wk[:, P * g : P * (g + 1)], dtype=f16),
                "wv": np.asarray(wv[:, P * g : P * (g + 1)], dtype=f16),
                "wo": np.asarray(wo[QW * g : QW * (g + 1), :], dtype=f16),
            }
        )
    return in_maps


def kernel(x, wq, wk, wv, wo):
    nc = _get_nc()
    in_maps = make_in_maps(x, wq, wk, wv, wo)
    res = run_bass_kernel_spmd(nc, in_maps, list(range(8)))
    parts = [res.results[c]["outT"] for c in range(8)]
    out = np.stack(
        [
            (parts[0] + parts[1] + parts[2] + parts[3]).T,
            (parts[4] + parts[5] + parts[6] + parts[7]).T,
        ]
    ).astype(np.float32)
    return out
